# revision 2
# baseline (speedup 1.0000x reference)
"""Capsule dynamic-routing kernel for Trainium2 (Bass/Tile), 8 NeuronCores.

Sharding: data-parallel over batch (B=64 -> 8 batches/core, grouped in 4
pairs of 2). W (64x256) is tiny and folded into per-iteration stationary
operands; no collectives are needed (pure SPMD).

The reference computes
    u_hat = u @ W                      # (N, 256), col c = k*16+d
    b=0; for i in 3: c = softmax_k(b); s[k,:] = sum_n c[k,n]*u_hat[n,kblk];
         out = squash(s); b += <out, u_hat>
u_hat is (B,N,256) = 512 MiB and never fits on chip.  We never materialize
it.  Since b_i = <sum_{j<i} out_j, u_hat>, with O = accumulated outputs and
Obd its (256,16) block-diagonal expansion:
    b_i[k,n] = <Wo[:,k], u[n,:]>   where Wo = W @ Obd   (64x16, tiny)
    s[k,d]   = sum_e G[k,e] W[e,k*16+d],  G[k,e] = sum_n c[k,n] u[n,e]
so each routing iteration only streams u (SBUF-resident, bf16) through the
PE array.

SBUF residents per core (bf16):
    ut[p] (128=2bx64e, N)      e-on-partitions ("transposed") for the b-pass
    un[p] (128=n-in-chunk, N)  n-on-partitions, chunk-major free, for G
b-pass per 128-n chunk:  bbT(128n, 32=2bx16k) = ut_chunk.T @ WoPair
softmax: free-dim (over k) ops at full 128-lane occupancy, no transposes
G-pass per chunk:        GT(128=2bx64e, 32) += un_chunk.T @ C_chunk
finalize per batch:      S(16,256) = G_b.T.T @ W ; mask diag blocks; squash.
"""

import numpy as np
from contextlib import ExitStack

import ml_dtypes

import concourse.bass as bass
import concourse.bacc as bacc
import concourse.tile as tile
import concourse.mybir as mybir
from concourse.bass_utils import run_bass_kernel_spmd

dt = mybir.dt
AFT = mybir.ActivationFunctionType
AXT = mybir.AxisListType
ALU = mybir.AluOpType

B, N_FULL, D = 64, 8192, 64
K, DCAP, KD = 16, 16, 256
NCORES = 8
NB = 8            # batches per core
NP = 4            # batch pairs per core
ROUTINGS = 3
EPS = 1e-7
CHUNK = 128       # n per contraction chunk
SUP = 16          # chunks per softmax super-chunk
SUBCOLS = 2048    # free columns per resident DMA sub-tile

U_DT = dt.bfloat16
U_NP = ml_dtypes.bfloat16


def build_program(n=N_FULL, reps=1, ablate=()):
    assert n % CHUNK == 0
    nch = n // CHUNK
    sup = min(SUP, nch)
    assert nch % sup == 0
    nsup = nch // sup
    subcols = min(SUBCOLS, n)
    nsub = n // subcols
    f32 = dt.float32

    nc = bacc.Bacc("TRN2", target_bir_lowering=False, debug=False)

    ut_d = nc.dram_tensor("ut", [NP, 128, n], U_DT, kind="ExternalInput").ap()
    un_d = nc.dram_tensor("un", [n // 512, 128, 2048],
                          U_DT, kind="ExternalInput").ap()
    wt_d = nc.dram_tensor("wt", [2, 128, D], U_DT, kind="ExternalInput").ap()
    wsb_d = nc.dram_tensor("wsb", [128, KD], U_DT, kind="ExternalInput").ap()
    mask_d = nc.dram_tensor("mask", [128, KD], f32, kind="ExternalInput").ap()
    ident_d = nc.dram_tensor("ident", [128, 128], f32, kind="ExternalInput").ap()
    out_d = nc.dram_tensor("out", [128, KD], f32, kind="ExternalOutput").ap()

    with tile.TileContext(nc) as tc, ExitStack() as ctx:
        consts = ctx.enter_context(tc.tile_pool(name="consts", bufs=1))
        resident = ctx.enter_context(tc.tile_pool(name="resident", bufs=1))
        work = ctx.enter_context(tc.tile_pool(name="work", bufs=1))
        c_pool = ctx.enter_context(tc.tile_pool(name="cpool", bufs=5))
        e_pool = ctx.enter_context(tc.tile_pool(name="epool", bufs=5))
        z_pool = ctx.enter_context(tc.tile_pool(name="zpool", bufs=8))
        ps_bb = ctx.enter_context(tc.tile_pool(name="psbb", bufs=3, space="PSUM"))
        ps_gt = ctx.enter_context(tc.tile_pool(name="psgt", bufs=1, space="PSUM"))

        # ---- constants ----
        wt_t = consts.tile([128, 2 * D], U_DT, tag="wt", name="wt")        # W.T halves
        for h in range(2):
            nc.sync.dma_start(out=wt_t[:, h * D:(h + 1) * D], in_=wt_d[h])
        wsb_t = consts.tile([128, KD], U_DT, tag="wsb", name="wsb")         # W stacked x2
        nc.sync.dma_start(out=wsb_t[:, :], in_=wsb_d[:, :])
        mask_t = consts.tile([128, KD], f32, tag="mask", name="mask")
        nc.sync.dma_start(out=mask_t[:, :], in_=mask_d[:, :])
        ident_t = consts.tile([128, 128], f32, tag="ident", name="ident")
        nc.sync.dma_start(out=ident_t[:, :], in_=ident_d[:, :])
        cu_t = consts.tile([128, 32], U_DT, tag="cu", name="cu")           # uniform c=1/16
        nc.vector.memset(cu_t[:, :], 1.0 / K)
        eps_t = consts.tile([128, 1], f32, tag="eps", name="eps")
        nc.vector.memset(eps_t[:, :], EPS)

        # ---- resident input copies ----
        nsubq = nch // 4  # un subtiles: 4 chunks x (4 pairs x 128) each
        un_t = [resident.tile([128, 2048], U_DT, tag=f"un{q}", name=f"un{q}")
                for q in range(nsubq)]
        ut_t = [[resident.tile([128, subcols], U_DT, tag=f"ut{p}_{q}", name=f"ut{p}_{q}")
                 for q in range(nsub)] for p in range(NP)]
        cpc = subcols // CHUNK  # chunks per sub-tile

        def un_chunk(p, j):
            base = (j % 4) * 512 + p * CHUNK
            return un_t[j // 4][:, base:base + CHUNK]

        def un_quad(j):
            return un_t[j // 4][:, (j % 4) * 512:(j % 4) * 512 + 512]

        def ut_chunk(p, j):
            return ut_t[p][j // cpc][:, (j % cpc) * CHUNK:(j % cpc + 1) * CHUNK]

        # ---- persistent work tiles ----
        o_acc = work.tile([128, KD], f32, tag="oacc", name="oacc")      # masked output accum
        s_all = work.tile([128, KD], f32, tag="sall", name="sall")
        sm = work.tile([128, KD], f32, tag="sm", name="sm")
        sq = work.tile([128, KD], f32, tag="sq", name="sq")
        o_fin = work.tile([128, KD], f32, tag="ofin", name="ofin")
        t1_sb = work.tile([128, 128], U_DT, tag="t1", name="t1")      # Obd halves
        t2_sb = work.tile([128, 128], U_DT, tag="t2", name="t2")
        wop = [work.tile([128, 32], U_DT, tag=f"wop{p}", name=f"wop{p}") for p in range(NP)]
        gt_sb = [work.tile([128, 32], U_DT, tag=f"gts{p}", name=f"gts{p}") for p in range(NP)]
        s2 = work.tile([128, 1], f32, tag="s2", name="s2")
        g0_sb = work.tile([32, 512], f32, tag="g0sb", name="g0sb")
        sc_a = work.tile([128, 1], f32, tag="sca", name="sca")
        sc_b = work.tile([128, 1], f32, tag="scb", name="scb")
        sc_c = work.tile([128, 1], f32, tag="scc", name="scc")
        sc_d = work.tile([128, 1], f32, tag="scd", name="scd")
        sc_e = work.tile([128, 1], f32, tag="sce", name="sce")

        gt_tiles = [ps_gt.tile([128, 32], f32, tag=f"gt{p}", name=f"gt{p}",
                       padded_shape=[128, 512]) for p in range(NP)]

        # cross-batch blocks of gt_sb / wop stay zero for the whole kernel
        for p in range(NP):
            nc.vector.memset(gt_sb[p][0:64, 16:32], 0.0)
            nc.vector.memset(gt_sb[p][64:128, 0:16], 0.0)
            nc.vector.memset(wop[p][0:64, 16:32], 0.0)
            nc.vector.memset(wop[p][64:128, 0:16], 0.0)

        def routing_pass(it):
            """b-pass (if it>0) + softmax + G-pass, accumulating gt_tiles."""
            for p in range(NP):
                for s in range(nsup):
                    if it == 0 or "nobb" in ablate:
                        def c_src(rel):
                            return cu_t[:, :]
                    else:
                        bb = ps_bb.tile([128, sup * 32], f32, tag="bb", name="bb",
                                        padded_shape=[128, 512])
                        for rel in range(sup):
                            j = s * sup + rel
                            nc.tensor.matmul(
                                bb[:, rel * 32:(rel + 1) * 32],
                                lhsT=ut_chunk(p, j), rhs=wop[p][:, :],
                                start=(rel == 0), stop=(rel == sup - 1))
                        e_t = e_pool.tile([128, sup * 32], f32, tag="e", name="e")
                        nc.scalar.activation(e_t[:, :], bb[:, :], AFT.Exp)
                        z_t = z_pool.tile([128, sup * 2], f32, tag="z", name="z")
                        nc.vector.reduce_sum(
                            z_t[:, :].rearrange("p (a b) -> p a b", b=2),
                            e_t[:, :].rearrange("p (a b c) -> p a b c", b=2, c=K),
                            axis=AXT.X)
                        zr_t = z_pool.tile([128, sup * 2], f32, tag="zr", name="zr")
                        nc.vector.reciprocal(zr_t[:, :], z_t[:, :])
                        c_t = c_pool.tile([128, sup * 32], U_DT, tag="c", name="c")
                        nc.vector.tensor_mul(
                            c_t[:, :].rearrange("p (a b c) -> p a b c", b=2, c=K),
                            e_t[:, :].rearrange("p (a b c) -> p a b c", b=2, c=K),
                            zr_t[:, :].rearrange("p (a b) -> p a b", b=2)
                                .broadcast_to([128, sup, 2, K]))

                        def c_src(rel, c_t=c_t):
                            return c_t[:, rel * 32:(rel + 1) * 32]
                    for rel in range(sup):
                        j = s * sup + rel
                        nc.tensor.matmul(
                            gt_tiles[p][:, :],
                            lhsT=un_chunk(p, j), rhs=c_src(rel),
                            start=(j == 0), stop=(j == nch - 1))

        def finalize(it):
            """gt -> s_all -> mask -> squash -> (o_acc | o_fin); update Wo."""
            for p in range(NP):
                # keep only the in-batch diagonal blocks of GT-pair;
                # cross-batch blocks are garbage and contract as zero
                nc.vector.tensor_copy(gt_sb[p][0:64, 0:16],
                                      gt_tiles[p][0:64, 0:16])
                nc.vector.tensor_copy(gt_sb[p][64:128, 16:32],
                                      gt_tiles[p][64:128, 16:32])
            for p in range(NP):
                sf = ps_bb.tile([32, KD], f32, tag="bb", name="sf",
                                padded_shape=[32, 512])
                nc.tensor.matmul(sf[:, :], lhsT=gt_sb[p][:, :],
                                 rhs=wsb_t[:, :], start=True, stop=True)
                # fused PSUM->SBUF copy + diagonal-block mask
                nc.vector.tensor_mul(sm[32 * p:32 * p + 32, :], sf[:, :],
                                     mask_t[32 * p:32 * p + 32, :])
            # squash: scale = s2/(1+s2)/sqrt(s2+EPS), s2 = sum_d sm^2 (row sum)
            nc.scalar.activation(sq[:, :], sm[:, :], AFT.Square,
                                 accum_out=s2[:, :])
            nc.vector.tensor_scalar_add(sc_a[:, :], s2[:, :], 1.0)
            nc.vector.reciprocal(sc_b[:, :], sc_a[:, :])
            nc.scalar.activation(sc_c[:, :], s2[:, :], AFT.Sqrt,
                                 bias=eps_t[:, :])
            nc.vector.reciprocal(sc_d[:, :], sc_c[:, :])
            nc.vector.tensor_mul(sc_e[:, :], sc_b[:, :], sc_d[:, :])
            nc.vector.tensor_mul(sc_e[:, :], sc_e[:, :], s2[:, :])
            tgt = o_fin if it == ROUTINGS - 1 else o_acc
            if it == 1:
                nc.vector.tensor_scalar_mul(o_fin[:, :], sm[:, :], sc_e[:, :])
                nc.vector.tensor_add(o_acc[:, :], o_acc[:, :], o_fin[:, :])
            else:
                nc.vector.tensor_scalar_mul(tgt[:, :], sm[:, :], sc_e[:, :])
            if it == ROUTINGS - 1:
                nc.sync.dma_start(out=out_d[:, :], in_=o_fin[:, :])
                return
            # Obd_b (256,16 block-diag of O_b) as columns of o_acc.T halves
            tps = []
            for h, t_sb in ((0, t1_sb), (1, t2_sb)):
                tp = ps_bb.tile([128, 128], f32, tag="bb", name="tp",
                                padded_shape=[128, 512])
                nc.tensor.transpose(tp[:, :], o_acc[:, h * 128:(h + 1) * 128],
                                    ident_t[:, :])
                nc.vector.tensor_copy(t_sb[:, :], tp[:, :])
                tps.append(tp)
            # Wo_b = W @ Obd_b, accumulated over the two 128-row halves of W.T
            wo = ps_bb.tile([64, NB * K], f32, tag="bb", name="wo",
                            padded_shape=[64, 512])
            for h2 in range(2):
                for b in range(NB):
                    nc.tensor.matmul(
                        wo[:, b * K:(b + 1) * K],
                        lhsT=wt_t[:, h2 * D:(h2 + 1) * D],
                        rhs=(t1_sb, t2_sb)[h2][:, b * K:(b + 1) * K],
                        start=(h2 == 0 and b == 0),
                        stop=(h2 == 1 and b == NB - 1))
            for b in range(NB):
                p, h = b // 2, b % 2
                nc.vector.tensor_copy(
                    wop[p][64 * h:64 * h + 64, 16 * h:16 * h + 16],
                    wo[:, b * K:(b + 1) * K])

        def rep_body():
            if "nodma" not in ablate:
                for q in range(nsubq):
                    nc.sync.dma_start(out=un_t[q][:, :], in_=un_d[q])
                for p in range(NP):
                    for q in range(nsub):
                        nc.sync.dma_start(
                            out=ut_t[p][q][:, :],
                            in_=ut_d[p, :, q * subcols:(q + 1) * subcols])
            if "nocompute" not in ablate:
                for it in range(ROUTINGS):
                    routing_pass(it)
                    finalize(it)
            else:
                nc.vector.memset(o_fin[:, :], 0.0)
                nc.sync.dma_start(out=out_d[:, :], in_=o_fin[:, :])

        if "nodma" in ablate:
            for q in range(nsubq):
                nc.vector.memset(un_t[q][:, 0:2], 0.0)
            for p in range(NP):
                for q in range(nsub):
                    nc.vector.memset(ut_t[p][q][:, 0:2], 0.0)
        if reps == 1:
            rep_body()
        else:
            with tc.For_i(0, reps, 1):
                rep_body()

    nc.compile()
    return nc


def host_inputs(u_shard, W):
    """Per-core DRAM inputs from an (8, N, 64) f32 batch shard + W (64, 256)."""
    n = u_shard.shape[1]
    ut = np.ascontiguousarray(
        u_shard.reshape(NP, 2, n, D).transpose(0, 1, 3, 2).reshape(NP, 128, n)
    ).astype(U_NP)
    un = np.ascontiguousarray(
        u_shard.reshape(NP, 2, n // 512, 4, CHUNK, D)
        .transpose(2, 4, 3, 0, 1, 5).reshape(n // 512, 128, 2048)
    ).astype(U_NP)
    return {"ut": ut, "un": un}


def host_consts(W):
    Wf = np.asarray(W, np.float32)
    wt = np.ascontiguousarray(Wf.T.reshape(2, 128, D)).astype(U_NP)
    wsb = np.ascontiguousarray(np.concatenate([Wf, Wf], 0)).astype(U_NP)
    base = np.kron(np.eye(K, dtype=np.float32), np.ones((1, DCAP), np.float32))
    mask = np.ascontiguousarray(np.tile(base, (NB, 1)))
    ident = np.eye(128, dtype=np.float32)
    return {"wt": wt, "wsb": wsb, "mask": mask, "ident": ident}


def extract_output(res_out):
    """(128, 256) masked f32 -> (8, 16, 16) squashed capsule outputs."""
    ar = np.arange(K)
    return res_out.reshape(NB, K, K, DCAP)[:, ar, ar, :]


_PROG_CACHE = {}


def _get_prog(n=N_FULL, reps=1):
    key = (n, reps)
    if key not in _PROG_CACHE:
        _PROG_CACHE[key] = build_program(n, reps)
    return _PROG_CACHE[key]


def kernel(u_vecs, W):
    u = np.ascontiguousarray(np.asarray(u_vecs, np.float32))
    assert u.shape == (B, N_FULL, D)
    nc = _get_prog()
    consts = host_consts(W)
    in_maps = [dict(consts, **host_inputs(u[c * NB:(c + 1) * NB], W))
               for c in range(NCORES)]
    res = run_bass_kernel_spmd(nc, in_maps, core_ids=list(range(NCORES)))
    return np.concatenate(
        [extract_output(res.results[c]["out"]) for c in range(NCORES)], axis=0
    ).astype(np.float32)



# revision 4
# speedup vs baseline: 1.0231x; 1.0231x over previous
"""Capsule dynamic-routing kernel for Trainium2 (Bass/Tile), 8 NeuronCores.

Sharding: data-parallel over batch (B=64 -> 8 batches/core, grouped in 4
pairs of 2). W (64x256) is tiny and folded into per-iteration stationary
operands; no collectives are needed (pure SPMD).

The reference computes
    u_hat = u @ W                      # (N, 256), col c = k*16+d
    b=0; for i in 3: c = softmax_k(b); s[k,:] = sum_n c[k,n]*u_hat[n,kblk];
         out = squash(s); b += <out, u_hat>
u_hat is (B,N,256) = 512 MiB and never fits on chip.  We never materialize
it.  Since b_i = <sum_{j<i} out_j, u_hat>, with O = accumulated outputs and
Obd its (256,16) block-diagonal expansion:
    b_i[k,n] = <Wo[:,k], u[n,:]>   where Wo = W @ Obd   (64x16, tiny)
    s[k,d]   = sum_e G[k,e] W[e,k*16+d],  G[k,e] = sum_n c[k,n] u[n,e]
so each routing iteration only streams u (SBUF-resident, bf16) through the
PE array.

SBUF residents per core (bf16):
    ut[p] (128=2bx64e, N)      e-on-partitions ("transposed") for the b-pass
    un[p] (128=n-in-chunk, N)  n-on-partitions, chunk-major free, for G
b-pass per 128-n chunk:  bbT(128n, 32=2bx16k) = ut_chunk.T @ WoPair
softmax: free-dim (over k) ops at full 128-lane occupancy, no transposes
G-pass per chunk:        GT(128=2bx64e, 32) += un_chunk.T @ C_chunk
finalize per batch:      S(16,256) = G_b.T.T @ W ; mask diag blocks; squash.
"""

import numpy as np
from contextlib import ExitStack

import ml_dtypes

import concourse.bass as bass
import concourse.bacc as bacc
import concourse.tile as tile
import concourse.mybir as mybir
from concourse.bass_utils import run_bass_kernel_spmd

dt = mybir.dt
AFT = mybir.ActivationFunctionType
AXT = mybir.AxisListType
ALU = mybir.AluOpType

B, N_FULL, D = 64, 8192, 64
K, DCAP, KD = 16, 16, 256
NCORES = 8
NB = 8            # batches per core
NP = 4            # batch pairs per core
ROUTINGS = 3
EPS = 1e-7
CHUNK = 128       # n per contraction chunk
SUP = 16          # chunks per softmax super-chunk
SUBCOLS = 2048    # free columns per resident DMA sub-tile

U_DT = dt.bfloat16
U_NP = ml_dtypes.bfloat16


def build_program(n=N_FULL, reps=1, ablate=()):
    assert n % CHUNK == 0
    nch = n // CHUNK
    sup = min(SUP, nch)
    assert nch % sup == 0
    nsup = nch // sup
    subcols = min(SUBCOLS, n)
    nsub = n // subcols
    f32 = dt.float32

    nc = bacc.Bacc("TRN2", target_bir_lowering=False, debug=False)

    ut_d = nc.dram_tensor("ut", [NP, 128, n], U_DT, kind="ExternalInput").ap()
    un_d = nc.dram_tensor("un", [n // 512, 128, 2048],
                          U_DT, kind="ExternalInput").ap()
    wt_d = nc.dram_tensor("wt", [2, 128, D], U_DT, kind="ExternalInput").ap()
    wsb_d = nc.dram_tensor("wsb", [128, KD], U_DT, kind="ExternalInput").ap()
    mask_d = nc.dram_tensor("mask", [128, KD], f32, kind="ExternalInput").ap()
    ident_d = nc.dram_tensor("ident", [128, 128], f32, kind="ExternalInput").ap()
    out_d = nc.dram_tensor("out", [128, KD], f32, kind="ExternalOutput").ap()

    with tile.TileContext(nc) as tc, ExitStack() as ctx:
        consts = ctx.enter_context(tc.tile_pool(name="consts", bufs=1))
        resident = ctx.enter_context(tc.tile_pool(name="resident", bufs=1))
        work = ctx.enter_context(tc.tile_pool(name="work", bufs=1))
        c_pool = ctx.enter_context(tc.tile_pool(name="cpool", bufs=5))
        e_pool = ctx.enter_context(tc.tile_pool(name="epool", bufs=5))
        z_pool = ctx.enter_context(tc.tile_pool(name="zpool", bufs=8))
        ps_bb = ctx.enter_context(tc.tile_pool(name="psbb", bufs=3, space="PSUM"))
        ps_gt = ctx.enter_context(tc.tile_pool(name="psgt", bufs=1, space="PSUM"))

        # ---- constants ----
        wt_t = consts.tile([128, 2 * D], U_DT, tag="wt", name="wt")        # W.T halves
        for h in range(2):
            nc.sync.dma_start(out=wt_t[:, h * D:(h + 1) * D], in_=wt_d[h])
        wsb_t = consts.tile([128, KD], U_DT, tag="wsb", name="wsb")         # W stacked x2
        nc.sync.dma_start(out=wsb_t[:, :], in_=wsb_d[:, :])
        mask_t = consts.tile([128, KD], f32, tag="mask", name="mask")
        nc.sync.dma_start(out=mask_t[:, :], in_=mask_d[:, :])
        ident_t = consts.tile([128, 128], f32, tag="ident", name="ident")
        nc.sync.dma_start(out=ident_t[:, :], in_=ident_d[:, :])
        cu_t = consts.tile([128, 32], U_DT, tag="cu", name="cu")           # uniform c=1/16
        nc.vector.memset(cu_t[:, :], 1.0 / K)
        eps_t = consts.tile([128, 1], f32, tag="eps", name="eps")
        nc.vector.memset(eps_t[:, :], EPS)

        # ---- resident input copies ----
        nsubq = nch // 4  # un subtiles: 4 chunks x (4 pairs x 128) each
        un_t = [resident.tile([128, 2048], U_DT, tag=f"un{q}", name=f"un{q}")
                for q in range(nsubq)]
        ut_t = [[resident.tile([128, subcols], U_DT, tag=f"ut{p}_{q}", name=f"ut{p}_{q}")
                 for q in range(nsub)] for p in range(NP)]
        cpc = subcols // CHUNK  # chunks per sub-tile

        def un_chunk(p, j):
            base = (j % 4) * 512 + p * CHUNK
            return un_t[j // 4][:, base:base + CHUNK]

        def un_quad(j):
            return un_t[j // 4][:, (j % 4) * 512:(j % 4) * 512 + 512]

        def ut_chunk(p, j):
            return ut_t[p][j // cpc][:, (j % cpc) * CHUNK:(j % cpc + 1) * CHUNK]

        # ---- persistent work tiles ----
        o_acc = work.tile([128, KD], f32, tag="oacc", name="oacc")      # masked output accum
        s_all = work.tile([128, KD], f32, tag="sall", name="sall")
        sm = work.tile([128, KD], f32, tag="sm", name="sm")
        sq = work.tile([128, KD], f32, tag="sq", name="sq")
        o_fin = work.tile([128, KD], f32, tag="ofin", name="ofin")
        t1_sb = work.tile([128, 128], U_DT, tag="t1", name="t1")      # Obd halves
        t2_sb = work.tile([128, 128], U_DT, tag="t2", name="t2")
        wop = [work.tile([128, 32], U_DT, tag=f"wop{p}", name=f"wop{p}") for p in range(NP)]
        gt_sb = [work.tile([128, 32], U_DT, tag=f"gts{p}", name=f"gts{p}") for p in range(NP)]
        s2 = work.tile([128, 1], f32, tag="s2", name="s2")
        g0_sb = work.tile([32, 512], f32, tag="g0sb", name="g0sb")
        sc_a = work.tile([128, 1], f32, tag="sca", name="sca")
        sc_b = work.tile([128, 1], f32, tag="scb", name="scb")
        sc_c = work.tile([128, 1], f32, tag="scc", name="scc")
        sc_d = work.tile([128, 1], f32, tag="scd", name="scd")
        sc_e = work.tile([128, 1], f32, tag="sce", name="sce")

        gt_tiles = [ps_gt.tile([128, 32], f32, tag=f"gt{p}", name=f"gt{p}",
                       padded_shape=[128, 512]) for p in range(NP)]

        # cross-batch blocks of gt_sb / wop stay zero for the whole kernel
        for p in range(NP):
            nc.vector.memset(gt_sb[p][0:64, 16:32], 0.0)
            nc.vector.memset(gt_sb[p][64:128, 0:16], 0.0)
            nc.vector.memset(wop[p][0:64, 16:32], 0.0)
            nc.vector.memset(wop[p][64:128, 0:16], 0.0)

        def routing_pass(it):
            """b-pass (if it>0) + softmax + G-pass, accumulating gt_tiles."""
            for p in range(NP):
                for s in range(nsup):
                    if it == 0 or "nobb" in ablate:
                        def c_src(rel):
                            return cu_t[:, :]
                    else:
                        bb = ps_bb.tile([128, sup * 32], f32, tag="bb", name="bb",
                                        padded_shape=[128, 512])
                        for rel in range(sup):
                            j = s * sup + rel
                            nc.tensor.matmul(
                                bb[:, rel * 32:(rel + 1) * 32],
                                lhsT=ut_chunk(p, j), rhs=wop[p][:, :],
                                start=(rel == 0), stop=(rel == sup - 1))
                        e_t = e_pool.tile([128, sup * 32], f32, tag="e", name="e")
                        nc.scalar.activation(e_t[:, :], bb[:, :], AFT.Exp)
                        z_t = z_pool.tile([128, sup * 2], f32, tag="z", name="z")
                        nc.vector.reduce_sum(
                            z_t[:, :].rearrange("p (a b) -> p a b", b=2),
                            e_t[:, :].rearrange("p (a b c) -> p a b c", b=2, c=K),
                            axis=AXT.X)
                        zr_t = z_pool.tile([128, sup * 2], f32, tag="zr", name="zr")
                        nc.vector.reciprocal(zr_t[:, :], z_t[:, :])
                        c_t = c_pool.tile([128, sup * 32], U_DT, tag="c", name="c")
                        nc.vector.tensor_mul(
                            c_t[:, :].rearrange("p (a b c) -> p a b c", b=2, c=K),
                            e_t[:, :].rearrange("p (a b c) -> p a b c", b=2, c=K),
                            zr_t[:, :].rearrange("p (a b) -> p a b", b=2)
                                .broadcast_to([128, sup, 2, K]))

                        def c_src(rel, c_t=c_t):
                            return c_t[:, rel * 32:(rel + 1) * 32]
                    for rel in range(sup):
                        j = s * sup + rel
                        nc.tensor.matmul(
                            gt_tiles[p][:, :],
                            lhsT=un_chunk(p, j), rhs=c_src(rel),
                            start=(j == 0), stop=(j == nch - 1))

        def finalize(it):
            """gt -> s_all -> mask -> squash -> (o_acc | o_fin); update Wo."""
            for p in range(NP):
                # keep only the in-batch diagonal blocks of GT-pair;
                # cross-batch blocks are garbage and contract as zero
                nc.vector.tensor_copy(gt_sb[p][0:64, 0:16],
                                      gt_tiles[p][0:64, 0:16])
                nc.vector.tensor_copy(gt_sb[p][64:128, 16:32],
                                      gt_tiles[p][64:128, 16:32])
            for p in range(NP):
                sf = ps_bb.tile([32, KD], f32, tag="bb", name="sf",
                                padded_shape=[32, 512])
                nc.tensor.matmul(sf[:, :], lhsT=gt_sb[p][:, :],
                                 rhs=wsb_t[:, :], start=True, stop=True)
                # fused PSUM->SBUF copy + diagonal-block mask
                nc.vector.tensor_mul(sm[32 * p:32 * p + 32, :], sf[:, :],
                                     mask_t[32 * p:32 * p + 32, :])
            # squash: scale = s2/(1+s2)/sqrt(s2+EPS), s2 = sum_d sm^2 (row sum)
            nc.scalar.activation(sq[:, :], sm[:, :], AFT.Square,
                                 accum_out=s2[:, :])
            nc.vector.tensor_scalar_add(sc_a[:, :], s2[:, :], 1.0)
            nc.vector.reciprocal(sc_b[:, :], sc_a[:, :])
            nc.scalar.activation(sc_c[:, :], s2[:, :], AFT.Sqrt,
                                 bias=eps_t[:, :])
            nc.vector.reciprocal(sc_d[:, :], sc_c[:, :])
            nc.vector.tensor_mul(sc_e[:, :], sc_b[:, :], sc_d[:, :])
            nc.vector.tensor_mul(sc_e[:, :], sc_e[:, :], s2[:, :])
            tgt = o_fin if it == ROUTINGS - 1 else o_acc
            if it == 1:
                nc.vector.tensor_scalar_mul(o_fin[:, :], sm[:, :], sc_e[:, :])
                nc.vector.tensor_add(o_acc[:, :], o_acc[:, :], o_fin[:, :])
            else:
                nc.vector.tensor_scalar_mul(tgt[:, :], sm[:, :], sc_e[:, :])
            if it == ROUTINGS - 1:
                nc.sync.dma_start(out=out_d[:, :], in_=o_fin[:, :])
                return
            # Obd_b (256,16 block-diag of O_b) as columns of o_acc.T halves
            tps = []
            for h, t_sb in ((0, t1_sb), (1, t2_sb)):
                tp = ps_bb.tile([128, 128], f32, tag="bb", name="tp",
                                padded_shape=[128, 512])
                nc.tensor.transpose(tp[:, :], o_acc[:, h * 128:(h + 1) * 128],
                                    ident_t[:, :])
                nc.vector.tensor_copy(t_sb[:, :], tp[:, :])
                tps.append(tp)
            # Wo_b = W @ Obd_b, accumulated over the two 128-row halves of W.T
            wo = ps_bb.tile([64, NB * K], f32, tag="bb", name="wo",
                            padded_shape=[64, 512])
            for h2 in range(2):
                for b in range(NB):
                    nc.tensor.matmul(
                        wo[:, b * K:(b + 1) * K],
                        lhsT=wt_t[:, h2 * D:(h2 + 1) * D],
                        rhs=(t1_sb, t2_sb)[h2][:, b * K:(b + 1) * K],
                        start=(h2 == 0 and b == 0),
                        stop=(h2 == 1 and b == NB - 1))
            for b in range(NB):
                p, h = b // 2, b % 2
                nc.vector.tensor_copy(
                    wop[p][64 * h:64 * h + 64, 16 * h:16 * h + 16],
                    wo[:, b * K:(b + 1) * K])

        def rep_body():
            if "nodma" not in ablate:
                for q in range(nsubq):
                    nc.sync.dma_start(out=un_t[q][:, :], in_=un_d[q])
                for p in range(NP):
                    for q in range(nsub):
                        nc.sync.dma_start(
                            out=ut_t[p][q][:, :],
                            in_=ut_d[p, :, q * subcols:(q + 1) * subcols])
            if "nocompute" not in ablate:
                for it in range(ROUTINGS):
                    routing_pass(it)
                    finalize(it)
            else:
                nc.vector.memset(o_fin[:, :], 0.0)
                nc.sync.dma_start(out=out_d[:, :], in_=o_fin[:, :])

        if "nodma" in ablate:
            for q in range(nsubq):
                nc.vector.memset(un_t[q][:, 0:2], 0.0)
            for p in range(NP):
                for q in range(nsub):
                    nc.vector.memset(ut_t[p][q][:, 0:2], 0.0)
        if reps <= 33:
            for rep in range(reps):
                rep_body()
                if rep < reps - 1:
                    tc.strict_bb_all_engine_barrier()
        else:
            with tc.For_i(0, reps, 1):
                rep_body()

    nc.compile()
    return nc


def host_inputs(u_shard, W):
    """Per-core DRAM inputs from an (8, N, 64) f32 batch shard + W (64, 256)."""
    n = u_shard.shape[1]
    ut = np.ascontiguousarray(
        u_shard.reshape(NP, 2, n, D).transpose(0, 1, 3, 2).reshape(NP, 128, n)
    ).astype(U_NP)
    un = np.ascontiguousarray(
        u_shard.reshape(NP, 2, n // 512, 4, CHUNK, D)
        .transpose(2, 4, 3, 0, 1, 5).reshape(n // 512, 128, 2048)
    ).astype(U_NP)
    return {"ut": ut, "un": un}


def host_consts(W):
    Wf = np.asarray(W, np.float32)
    wt = np.ascontiguousarray(Wf.T.reshape(2, 128, D)).astype(U_NP)
    wsb = np.ascontiguousarray(np.concatenate([Wf, Wf], 0)).astype(U_NP)
    base = np.kron(np.eye(K, dtype=np.float32), np.ones((1, DCAP), np.float32))
    mask = np.ascontiguousarray(np.tile(base, (NB, 1)))
    ident = np.eye(128, dtype=np.float32)
    return {"wt": wt, "wsb": wsb, "mask": mask, "ident": ident}


def extract_output(res_out):
    """(128, 256) masked f32 -> (8, 16, 16) squashed capsule outputs."""
    ar = np.arange(K)
    return res_out.reshape(NB, K, K, DCAP)[:, ar, ar, :]


_PROG_CACHE = {}


def _get_prog(n=N_FULL, reps=1):
    key = (n, reps)
    if key not in _PROG_CACHE:
        _PROG_CACHE[key] = build_program(n, reps)
    return _PROG_CACHE[key]


def kernel(u_vecs, W):
    u = np.ascontiguousarray(np.asarray(u_vecs, np.float32))
    assert u.shape == (B, N_FULL, D)
    nc = _get_prog()
    consts = host_consts(W)
    in_maps = [dict(consts, **host_inputs(u[c * NB:(c + 1) * NB], W))
               for c in range(NCORES)]
    res = run_bass_kernel_spmd(nc, in_maps, core_ids=list(range(NCORES)))
    return np.concatenate(
        [extract_output(res.results[c]["out"]) for c in range(NCORES)], axis=0
    ).astype(np.float32)



# revision 26
# speedup vs baseline: 1.3817x; 1.3505x over previous
"""Capsule dynamic-routing kernel for Trainium2 (Bass/Tile), 8 NeuronCores.

Sharding: data-parallel over batch (B=64 -> 8 batches/core, grouped in 4
pairs of 2). W (64x256) is tiny and folded into per-iteration stationary
operands; no collectives are needed (pure SPMD).

The reference computes
    u_hat = u @ W                      # (N, 256), col c = k*16+d
    b=0; for i in 3: c = softmax_k(b); s[k,:] = sum_n c[k,n]*u_hat[n,kblk];
         out = squash(s); b += <out, u_hat>
u_hat is (B,N,256) = 512 MiB and never fits on chip.  We never materialize
it.  Since b_i = <sum_{j<i} out_j, u_hat>, with O = accumulated outputs and
Obd its (256,16) block-diagonal expansion:
    b_i[k,n] = <Wo[:,k], u[n,:]>   where Wo = W @ Obd   (64x16, tiny)
    s[k,d]   = sum_e G[k,e] W[e,k*16+d],  G[k,e] = sum_n c[k,n] u[n,e]
so each routing iteration only streams u (SBUF-resident) through the
PE array.

v2 (PE-LDW-bound; DMA is not the bottleneck):
 - un (n-on-partitions, bf16, 8 MiB) feeds all G-passes; DMA'd FIRST so
   iteration 0's G-pass (uniform c) pipelines with its arrival.
 - ut8 (e-on-partitions, fp8-e4m3, 4 MiB) feeds the b-passes only; b-side
   fp8 noise costs ~4e-3 rel err (vs 2e-2 budget) and halves that DMA +
   speeds LDWEIGHTS.  wop stays bf16 (mixed fp8xbf16 matmul).
 - loops run super-chunk-outer / pair-inner so all 4 batch-pairs consume
   DMA tiles in arrival order.
 - softmax's exp output is bf16 (2x DVE for the z-reduce and c-mul).
 - finalize never touches ACT Sqrt/Square (they'd force act-table reloads
   around Exp): s2 via one fused DVE tensor_tensor_reduce, rsqrt via
   int-bit-trick seed + 2 Newton steps on DVE.

SBUF residents per core:
    un[q]  [128, 2048] bf16 x16   n-on-partitions, (chunk,pair,2b,e) free
    ut8[p][q] [128, 2048] fp8 x16 (2b,e)-on-partitions, n free
b-pass per 128-n chunk:  bb(128n, 32=2bx16k) = ut8_chunk.T @ wop  (PSUM f32)
softmax: free-dim (over k) ops at full 128-lane occupancy
G-pass per chunk:        GT(128=2bx64e, 32) += un_chunk.T @ C_chunk
finalize per batch:      S(16,256) = G_b.T.T @ W ; mask diag blocks; squash.
"""

import numpy as np
from contextlib import ExitStack

import ml_dtypes

import concourse.bass as bass
import concourse.bacc as bacc
import concourse.tile as tile
import concourse.mybir as mybir
from concourse.bass_utils import run_bass_kernel_spmd

dt = mybir.dt
AFT = mybir.ActivationFunctionType
AXT = mybir.AxisListType
ALU = mybir.AluOpType

B, N_FULL, D = 64, 8192, 64
K, DCAP, KD = 16, 16, 256
NCORES = 8
NB = 8            # batches per core
NP = 4            # batch pairs per core
ROUTINGS = 3
EPS = 1e-7
CHUNK = 128       # n per contraction chunk
SUP = 16          # chunks per softmax super-chunk
SUBCOLS = 2048    # free columns per resident DMA sub-tile

U_DT = dt.bfloat16
U_NP = ml_dtypes.bfloat16
U8_DT = dt.float8e4
U8_NP = ml_dtypes.float8_e4m3


def build_program(n=N_FULL, reps=1, ablate=(), u8=True, sp_order=True,
                  pipe_ahead=2, z16=True, sup_cols=SUP, bb_bufs=3):
    assert n % CHUNK == 0
    nch = n // CHUNK
    sup = min(sup_cols, nch)
    assert nch % sup == 0
    nsup = nch // sup
    subcols = min(SUBCOLS, n)
    nsub = n // subcols
    f32 = dt.float32

    nc = bacc.Bacc("TRN2", target_bir_lowering=False, debug=False)

    un_d = nc.dram_tensor("un", [max(1, n // 512), 128, min(2048, 16 * n // 4)],
                          U_DT, kind="ExternalInput").ap()
    UT_DT = U8_DT if u8 else U_DT
    ut8_d = nc.dram_tensor("ut8" if u8 else "ut", [NP, 128, n], UT_DT,
                           kind="ExternalInput").ap()
    wt_d = nc.dram_tensor("wt", [2, 128, D], U_DT, kind="ExternalInput").ap()
    wsb_d = nc.dram_tensor("wsb", [128, KD], U_DT, kind="ExternalInput").ap()
    mask_d = nc.dram_tensor("mask", [128, KD], f32, kind="ExternalInput").ap()
    ident_d = nc.dram_tensor("ident", [128, 128], f32, kind="ExternalInput").ap()
    out_d = nc.dram_tensor("out", [128, KD], f32, kind="ExternalOutput").ap()

    with tile.TileContext(nc) as tc, ExitStack() as ctx:
        consts = ctx.enter_context(tc.tile_pool(name="consts", bufs=1))
        resident = ctx.enter_context(tc.tile_pool(name="resident", bufs=1))
        work = ctx.enter_context(tc.tile_pool(name="work", bufs=1))
        c_pool = ctx.enter_context(tc.tile_pool(name="cpool", bufs=5))
        e_pool = ctx.enter_context(tc.tile_pool(name="epool", bufs=5))
        z_pool = ctx.enter_context(tc.tile_pool(name="zpool", bufs=8))
        ps_bb = ctx.enter_context(tc.tile_pool(name="psbb", bufs=bb_bufs, space="PSUM"))
        ps_gt = ctx.enter_context(tc.tile_pool(name="psgt", bufs=1, space="PSUM"))

        # ---- constants ----
        wt_t = consts.tile([128, 2 * D], U_DT, tag="wt", name="wt")        # W.T halves
        for h in range(2):
            nc.sync.dma_start(out=wt_t[:, h * D:(h + 1) * D], in_=wt_d[h])
        wsb_t = consts.tile([128, KD], U_DT, tag="wsb", name="wsb")         # W stacked x2
        nc.sync.dma_start(out=wsb_t[:, :], in_=wsb_d[:, :])
        mask_t = consts.tile([128, KD], f32, tag="mask", name="mask")
        nc.sync.dma_start(out=mask_t[:, :], in_=mask_d[:, :])
        ident_t = consts.tile([128, 128], f32, tag="ident", name="ident")
        nc.sync.dma_start(out=ident_t[:, :], in_=ident_d[:, :])
        cu_t = consts.tile([128, 32], U_DT, tag="cu", name="cu")           # uniform c=1/16
        nc.vector.memset(cu_t[:, :], 1.0 / K)

        # ---- resident input copies ----
        nsubq = nch // 4  # un subtiles: 4 chunks x (4 pairs x 128) each
        un_t = [resident.tile([128, 2048], U_DT, tag=f"un{q}", name=f"un{q}")
                for q in range(nsubq)]
        ut8_t = [[resident.tile([128, subcols], UT_DT, tag=f"ut{p}_{q}",
                                name=f"ut{p}_{q}")
                  for q in range(nsub)] for p in range(NP)]
        cpc = subcols // CHUNK  # chunks per sub-tile

        def un_chunk(p, j):
            base = (j % 4) * 512 + p * CHUNK
            return un_t[j // 4][:, base:base + CHUNK]

        def ut8_chunk(p, j):
            return ut8_t[p][j // cpc][:, (j % cpc) * CHUNK:(j % cpc + 1) * CHUNK]

        # ---- persistent work tiles ----
        o_acc = work.tile([128, KD], f32, tag="oacc", name="oacc")      # masked output accum
        sm = work.tile([128, KD], f32, tag="sm", name="sm")
        sq = work.tile([128, KD], f32, tag="sq", name="sq")
        o_fin = work.tile([128, KD], f32, tag="ofin", name="ofin")
        t1_sb = work.tile([128, 128], U_DT, tag="t1", name="t1")      # Obd halves
        t2_sb = work.tile([128, 128], U_DT, tag="t2", name="t2")
        wop = [work.tile([128, 32], U_DT, tag=f"wop{p}", name=f"wop{p}")
               for p in range(NP)]
        gt_sb = [work.tile([128, 32], U_DT, tag=f"gts{p}", name=f"gts{p}")
                 for p in range(NP)]
        s2 = work.tile([128, 1], f32, tag="s2", name="s2")
        sc_a = work.tile([128, 1], f32, tag="sca", name="sca")
        sc_b = work.tile([128, 1], f32, tag="scb", name="scb")
        sc_e = work.tile([128, 1], f32, tag="sce", name="sce")
        xe = work.tile([128, 1], f32, tag="xe", name="xe")
        xh = work.tile([128, 1], f32, tag="xh", name="xh")
        yr = work.tile([128, 1], f32, tag="yr", name="yr")
        q1 = work.tile([128, 1], f32, tag="q1", name="q1")

        gt_tiles = [ps_gt.tile([128, 32], f32, tag=f"gt{p}", name=f"gt{p}",
                       padded_shape=[128, 512]) for p in range(NP)]

        # cross-batch blocks of gt_sb / wop stay zero for the whole kernel
        for p in range(NP):
            nc.vector.memset(gt_sb[p][0:64, 16:32], 0.0)
            nc.vector.memset(gt_sb[p][64:128, 0:16], 0.0)
            nc.vector.memset(wop[p][0:64, 16:32], 0.0)
            nc.vector.memset(wop[p][64:128, 0:16], 0.0)

        def routing_pass(it):
            """b-pass (if it>0) + softmax + G-pass, accumulating gt_tiles.

            Super-chunk-outer, pair-inner: consumes DMA tiles in order.
            The b-pass runs PIPE_AHEAD (s,p)-groups ahead of the G-pass so
            the PE never stalls on the ACT->DVE softmax latency of the
            current group (PE queues are strictly in-order)."""
            sp = ([(s, p) for s in range(nsup) for p in range(NP)] if sp_order
                  else [(s, p) for p in range(NP) for s in range(nsup)])
            uniform = it == 0 or "nobb" in ablate
            PIPE_AHEAD = 0 if uniform else pipe_ahead
            c_srcs = {}

            def emit_b(s, p):
                if uniform:
                    c_srcs[(s, p)] = lambda rel: cu_t[:, :]
                    return
                bb = ps_bb.tile([128, sup * 32], f32, tag="bb", name="bb",
                                padded_shape=[128, max(512, sup * 32)])
                for rel in range(sup):
                    j = s * sup + rel
                    nc.tensor.matmul(
                        bb[:, rel * 32:(rel + 1) * 32],
                        lhsT=ut8_chunk(p, j), rhs=wop[p][:, :],
                        start=(rel == 0), stop=(rel == sup - 1))
                if "nosm" in ablate:
                    c_srcs[(s, p)] = lambda rel: cu_t[:, :]
                    return
                e_t = e_pool.tile([128, sup * 32], U_DT, tag="e", name="e")
                nc.scalar.activation(e_t[:, :], bb[:, :], AFT.Exp)
                # bf16 z keeps every reduce operand 2-byte -> DVE 2x packed
                z_t = z_pool.tile([128, sup * 2], U_DT if z16 else f32,
                                  tag="z", name="z")
                with nc.allow_low_precision("z is a 16-term exp sum; c is "
                                            "bf16 downstream anyway"):
                    nc.vector.reduce_sum(
                        z_t[:, :].rearrange("p (a b) -> p a b", b=2),
                        e_t[:, :].rearrange("p (a b c) -> p a b c", b=2, c=K),
                        axis=AXT.X)
                zr_t = z_pool.tile([128, sup * 2], f32, tag="zr", name="zr")
                if z16:
                    # f32 staging for the approx reciprocal's bit-trick seed
                    # (cast on ACT -- it has slack, DVE is the busy engine)
                    zf_t = z_pool.tile([128, sup * 2], f32, tag="zf", name="zf")
                    nc.scalar.copy(zf_t[:, :], z_t[:, :])
                    nc.vector.reciprocal_approx_fast(zr_t[:, :], zf_t[:, :])
                else:
                    nc.vector.reciprocal(zr_t[:, :], z_t[:, :])
                c_t = c_pool.tile([128, sup * 32], U_DT, tag="c", name="c")
                nc.vector.tensor_mul(
                    c_t[:, :].rearrange("p (a b c) -> p a b c", b=2, c=K),
                    e_t[:, :].rearrange("p (a b c) -> p a b c", b=2, c=K),
                    zr_t[:, :].rearrange("p (a b) -> p a b", b=2)
                        .broadcast_to([128, sup, 2, K]))
                c_srcs[(s, p)] = lambda rel, c_t=c_t: \
                    c_t[:, rel * 32:(rel + 1) * 32]

            def emit_g(s, p):
                c_src = c_srcs.pop((s, p))
                for rel in range(sup):
                    j = s * sup + rel
                    if "nog" in ablate and j not in (0, nch - 1):
                        continue
                    nc.tensor.matmul(
                        gt_tiles[p][:, :],
                        lhsT=un_chunk(p, j), rhs=c_src(rel),
                        start=(j == 0), stop=(j == nch - 1))

            for i, (s, p) in enumerate(sp):
                emit_b(s, p)
                if i >= PIPE_AHEAD:
                    emit_g(*sp[i - PIPE_AHEAD])
            for i in range(len(sp) - PIPE_AHEAD, len(sp)):
                emit_g(*sp[i])

        def squash_scale():
            """sc_e = s2/(1+s2)/sqrt(s2+EPS), all on DVE (no ACT table)."""
            nc.vector.tensor_scalar_add(sc_a[:, :], s2[:, :], 1.0)
            nc.vector.reciprocal(sc_b[:, :], sc_a[:, :])
            nc.vector.tensor_scalar_add(xe[:, :], s2[:, :], EPS)
            nc.vector.tensor_scalar_mul(xh[:, :], xe[:, :], 0.5)
            # rsqrt(xe): bit-trick seed + 2 Newton steps
            yi = yr[:, :].bitcast(dt.int32)
            nc.vector.tensor_scalar(yi, xe[:, :].bitcast(dt.int32),
                                    1, -1, ALU.logical_shift_right,
                                    ALU.bitwise_xor)
            nc.vector.tensor_scalar_add(yi, yi, 0x5f3759e0)
            for _ in range(2):
                nc.vector.tensor_mul(q1[:, :], yr[:, :], yr[:, :])
                nc.vector.tensor_mul(q1[:, :], q1[:, :], xh[:, :])
                nc.vector.tensor_scalar(q1[:, :], q1[:, :], -1.0, 1.5,
                                        ALU.mult, ALU.add)
                nc.vector.tensor_mul(yr[:, :], yr[:, :], q1[:, :])
            nc.vector.tensor_mul(sc_e[:, :], sc_b[:, :], yr[:, :])
            nc.vector.tensor_mul(sc_e[:, :], sc_e[:, :], s2[:, :])

        def finalize(it):
            """gt -> sm -> mask -> squash -> (o_acc | o_fin); update Wo."""
            for p in range(NP):
                # keep only the in-batch diagonal blocks of GT-pair;
                # cross-batch blocks are garbage and contract as zero
                nc.vector.tensor_copy(gt_sb[p][0:64, 0:16],
                                      gt_tiles[p][0:64, 0:16])
                nc.vector.tensor_copy(gt_sb[p][64:128, 16:32],
                                      gt_tiles[p][64:128, 16:32])
            for p in range(NP):
                sf = ps_bb.tile([32, KD], f32, tag="bb", name="sf",
                                padded_shape=[32, 512])
                nc.tensor.matmul(sf[:, :], lhsT=gt_sb[p][:, :],
                                 rhs=wsb_t[:, :], start=True, stop=True)
                # fused PSUM->SBUF copy + diagonal-block mask
                nc.vector.tensor_mul(sm[32 * p:32 * p + 32, :], sf[:, :],
                                     mask_t[32 * p:32 * p + 32, :])
            # s2 = rowsum(sm^2) on DVE (keeps ACT on the Exp table)
            nc.vector.tensor_mul(sq[:, :], sm[:, :], sm[:, :])
            nc.vector.reduce_sum(s2[:, :], sq[:, :], axis=AXT.X)
            squash_scale()
            tgt = o_fin if it == ROUTINGS - 1 else o_acc
            if it == 1:
                nc.vector.tensor_scalar_mul(o_fin[:, :], sm[:, :], sc_e[:, :])
                nc.vector.tensor_add(o_acc[:, :], o_acc[:, :], o_fin[:, :])
            else:
                nc.vector.tensor_scalar_mul(tgt[:, :], sm[:, :], sc_e[:, :])
            if it == ROUTINGS - 1:
                nc.sync.dma_start(out=out_d[:, :], in_=o_fin[:, :])
                return
            # Obd_b (256,16 block-diag of O_b) as columns of o_acc.T halves
            for h, t_sb in ((0, t1_sb), (1, t2_sb)):
                tp = ps_bb.tile([128, 128], f32, tag="bb", name="tp",
                                padded_shape=[128, 512])
                nc.tensor.transpose(tp[:, :], o_acc[:, h * 128:(h + 1) * 128],
                                    ident_t[:, :])
                nc.vector.tensor_copy(t_sb[:, :], tp[:, :])
            # Wo_b = W @ Obd_b, accumulated over the two 128-row halves of W.T
            wo = ps_bb.tile([64, NB * K], f32, tag="bb", name="wo",
                            padded_shape=[64, 512])
            for h2 in range(2):
                for b in range(NB):
                    nc.tensor.matmul(
                        wo[:, b * K:(b + 1) * K],
                        lhsT=wt_t[:, h2 * D:(h2 + 1) * D],
                        rhs=(t1_sb, t2_sb)[h2][:, b * K:(b + 1) * K],
                        start=(h2 == 0 and b == 0),
                        stop=(h2 == 1 and b == NB - 1))
            for b in range(NB):
                p, h = b // 2, b % 2
                nc.vector.tensor_copy(
                    wop[p][64 * h:64 * h + 64, 16 * h:16 * h + 16],
                    wo[:, b * K:(b + 1) * K])

        def rep_body():
            if "nodma" not in ablate:
                for q in range(nsubq):
                    nc.sync.dma_start(out=un_t[q][:, :], in_=un_d[q])
                for q in range(nsub):
                    for p in range(NP):
                        nc.sync.dma_start(
                            out=ut8_t[p][q][:, :],
                            in_=ut8_d[p, :, q * subcols:(q + 1) * subcols])
            if "dmawait" in ablate:
                # force completion: read one column of every DMA'd tile
                sink = work.tile([128, 64], f32, tag="sink", name="sink")
                for q in range(nsubq):
                    nc.vector.tensor_copy(sink[:, q:q + 1], un_t[q][:, 0:1])
                for p in range(NP):
                    for q in range(nsub):
                        nc.vector.tensor_copy(
                            sink[:, 32 + 4 * p + q:33 + 4 * p + q],
                            ut8_t[p][q][:, 0:1])
                nc.sync.dma_start(out=out_d[:, 0:64], in_=sink[:, :])
            elif "nocompute" not in ablate:
                for it in range(ROUTINGS):
                    routing_pass(it)
                    finalize(it)
            else:
                nc.vector.memset(o_fin[:, :], 0.0)
                nc.sync.dma_start(out=out_d[:, :], in_=o_fin[:, :])

        if "nodma" in ablate:
            for q in range(nsubq):
                nc.vector.memset(un_t[q][:, 0:2], 0.0)
            for p in range(NP):
                for q in range(nsub):
                    nc.vector.memset(ut8_t[p][q][:, 0:2], 0.0)
        if reps <= 33:
            for rep in range(reps):
                rep_body()
                if rep < reps - 1:
                    tc.strict_bb_all_engine_barrier()
        else:
            with tc.For_i(0, reps, 1):
                rep_body()

    nc.compile()
    return nc


def host_inputs(u_shard, W):
    """Per-core DRAM inputs from an (8, N, 64) f32 batch shard + W (64, 256)."""
    n = u_shard.shape[1]
    ut = np.ascontiguousarray(
        u_shard.reshape(NP, 2, n, D).transpose(0, 1, 3, 2).reshape(NP, 128, n))
    un = np.ascontiguousarray(
        u_shard.reshape(NP, 2, n // 512, 4, CHUNK, D)
        .transpose(2, 4, 3, 0, 1, 5).reshape(n // 512, 128, 2048)
    ).astype(U_NP)
    return {"ut8": ut.astype(U8_NP), "ut": ut.astype(U_NP), "un": un}


def host_consts(W):
    Wf = np.asarray(W, np.float32)
    wt = np.ascontiguousarray(Wf.T.reshape(2, 128, D)).astype(U_NP)
    wsb = np.ascontiguousarray(np.concatenate([Wf, Wf], 0)).astype(U_NP)
    base = np.kron(np.eye(K, dtype=np.float32), np.ones((1, DCAP), np.float32))
    mask = np.ascontiguousarray(np.tile(base, (NB, 1)))
    ident = np.eye(128, dtype=np.float32)
    return {"wt": wt, "wsb": wsb, "mask": mask, "ident": ident}


def extract_output(res_out):
    """(128, 256) masked f32 -> (8, 16, 16) squashed capsule outputs."""
    ar = np.arange(K)
    return res_out.reshape(NB, K, K, DCAP)[:, ar, ar, :]


_PROG_CACHE = {}


def _get_prog(n=N_FULL, reps=1):
    key = (n, reps)
    if key not in _PROG_CACHE:
        _PROG_CACHE[key] = build_program(n, reps)
    return _PROG_CACHE[key]


def kernel(u_vecs, W):
    u = np.ascontiguousarray(np.asarray(u_vecs, np.float32))
    assert u.shape == (B, N_FULL, D)
    nc = _get_prog()
    consts = host_consts(W)
    in_maps = [dict(consts, **host_inputs(u[c * NB:(c + 1) * NB], W))
               for c in range(NCORES)]
    res = run_bass_kernel_spmd(nc, in_maps, core_ids=list(range(NCORES)))
    return np.concatenate(
        [extract_output(res.results[c]["out"]) for c in range(NCORES)], axis=0
    ).astype(np.float32)


# revision 29
# speedup vs baseline: 1.7771x; 1.2862x over previous
"""Capsule dynamic-routing kernel for Trainium2 (Bass/Tile), 8 NeuronCores.

Sharding: data-parallel over batch (B=64 -> 8 batches/core, grouped in 4
pairs of 2). W (64x256) is tiny and folded into per-iteration stationary
operands; no collectives are needed (pure SPMD).

The reference computes
    u_hat = u @ W                      # (N, 256), col c = k*16+d
    b=0; for i in 3: c = softmax_k(b); s[k,:] = sum_n c[k,n]*u_hat[n,kblk];
         out = squash(s); b += <out, u_hat>
u_hat is (B,N,256) = 512 MiB and never fits on chip.  We never materialize
it.  Since b_i = <sum_{j<i} out_j, u_hat>, with O = accumulated outputs and
Obd its (256,16) block-diagonal expansion:
    b_i[k,n] = <Wo[:,k], u[n,:]>   where Wo = W @ Obd   (64x16, tiny)
    s[k,d]   = sum_e G[k,e] W[e,k*16+d],  G[k,e] = sum_n c[k,n] u[n,e]
so each routing iteration only streams u (SBUF-resident) through the
PE array.

v2 (measured on HW via same-session interleaved A/B; ~20% under v1):
 - un (n-on-partitions, bf16, 8 MiB) feeds all G-passes; DMA'd FIRST so
   iteration 0's G-pass (uniform c) pipelines with its arrival.
 - ut8 (e-on-partitions, fp8-e4m3, 4 MiB) feeds the b-passes only; b-side
   fp8 noise is iid across n and washes out (~4e-3 rel err vs 2e-2
   budget); it halves that DMA and speeds LDWEIGHTS.  wop stays bf16
   (mixed fp8 x bf16 matmul works on HW).  NOTE: fp8 on the G side is NOT
   safe -- G0/G2 especially (out-noise is low-rank/systematic and does
   not average out; measured 3e-2).
 - loops run super-chunk-outer / pair-inner so all 4 batch-pairs consume
   DMA tiles in arrival order; the b-pass+softmax runs PIPE_AHEAD=2
   groups ahead of the G-pass so the in-order PE queue never waits on
   the ACT->DVE softmax chain (-11us).
 - softmax: exp out and the z sums are bf16 with stride-1 innermost so
   the DVE z-reduce runs in 2x packed mode (-10us); 1/z via
   reciprocal_approx_fast (f32 staging cast done on ACT).
 - finalize never touches ACT Sqrt/Square (they'd force act-table
   reloads around Exp -- LoadActFuncSet is ~1.3us each): s2 via DVE
   mul+reduce, rsqrt via int-bit-trick seed + 2 Newton steps on DVE.

SBUF residents per core:
    un[q]  [128, 2048] bf16 x16   n-on-partitions, (chunk,pair,2b,e) free
    ut8[p][q] [128, 2048] fp8 x16 (2b,e)-on-partitions, n free
b-pass per 128-n chunk:  bb(128n, 32=2bx16k) = ut8_chunk.T @ wop  (PSUM f32)
softmax: free-dim (over k) ops at full 128-lane occupancy
G-pass per chunk:        GT(128=2bx64e, 32) += un_chunk.T @ C_chunk
finalize per batch:      S(16,256) = G_b.T.T @ W ; mask diag blocks; squash.
"""

import numpy as np
from contextlib import ExitStack

import ml_dtypes

import concourse.bass as bass
import concourse.bacc as bacc
import concourse.tile as tile
import concourse.mybir as mybir
from concourse.bass_utils import run_bass_kernel_spmd

dt = mybir.dt
AFT = mybir.ActivationFunctionType
AXT = mybir.AxisListType
ALU = mybir.AluOpType

B, N_FULL, D = 64, 8192, 64
K, DCAP, KD = 16, 16, 256
NCORES = 8
NB = 8            # batches per core
NP = 4            # batch pairs per core
ROUTINGS = 3
EPS = 1e-7
CHUNK = 128       # n per contraction chunk
SUP = 16          # chunks per softmax super-chunk
SUBCOLS = 2048    # free columns per resident DMA sub-tile

U_DT = dt.bfloat16
U_NP = ml_dtypes.bfloat16
U8_DT = dt.float8e4
U8_NP = ml_dtypes.float8_e4m3


def build_program(n=N_FULL, reps=1, ablate=(), u8=True, sp_order=True,
                  pipe_ahead=2, z16=True, sup_cols=SUP, bb_bufs=3):
    assert n % CHUNK == 0
    nch = n // CHUNK
    sup = min(sup_cols, nch)
    assert nch % sup == 0
    nsup = nch // sup
    subcols = min(SUBCOLS, n)
    nsub = n // subcols
    f32 = dt.float32

    nc = bacc.Bacc("TRN2", target_bir_lowering=False, debug=False)

    un_d = nc.dram_tensor("un", [max(1, n // 512), 128, min(2048, 16 * n // 4)],
                          U_DT, kind="ExternalInput").ap()
    UT_DT = U8_DT if u8 else U_DT
    ut8_d = nc.dram_tensor("ut8" if u8 else "ut", [NP, 128, n], UT_DT,
                           kind="ExternalInput").ap()
    wt_d = nc.dram_tensor("wt", [2, 128, D], U_DT, kind="ExternalInput").ap()
    wsb_d = nc.dram_tensor("wsb", [128, KD], U_DT, kind="ExternalInput").ap()
    mask_d = nc.dram_tensor("mask", [128, KD], f32, kind="ExternalInput").ap()
    ident_d = nc.dram_tensor("ident", [128, 128], f32, kind="ExternalInput").ap()
    out_d = nc.dram_tensor("out", [128, KD], f32, kind="ExternalOutput").ap()

    with tile.TileContext(nc) as tc, ExitStack() as ctx:
        consts = ctx.enter_context(tc.tile_pool(name="consts", bufs=1))
        resident = ctx.enter_context(tc.tile_pool(name="resident", bufs=1))
        work = ctx.enter_context(tc.tile_pool(name="work", bufs=1))
        c_pool = ctx.enter_context(tc.tile_pool(name="cpool", bufs=5))
        e_pool = ctx.enter_context(tc.tile_pool(name="epool", bufs=5))
        z_pool = ctx.enter_context(tc.tile_pool(name="zpool", bufs=8))
        ps_bb = ctx.enter_context(tc.tile_pool(name="psbb", bufs=bb_bufs, space="PSUM"))
        ps_gt = ctx.enter_context(tc.tile_pool(name="psgt", bufs=1, space="PSUM"))

        # ---- constants ----
        wt_t = consts.tile([128, 2 * D], U_DT, tag="wt", name="wt")        # W.T halves
        for h in range(2):
            nc.sync.dma_start(out=wt_t[:, h * D:(h + 1) * D], in_=wt_d[h])
        wsb_t = consts.tile([128, KD], U_DT, tag="wsb", name="wsb")         # W stacked x2
        nc.sync.dma_start(out=wsb_t[:, :], in_=wsb_d[:, :])
        mask_t = consts.tile([128, KD], f32, tag="mask", name="mask")
        nc.sync.dma_start(out=mask_t[:, :], in_=mask_d[:, :])
        ident_t = consts.tile([128, 128], f32, tag="ident", name="ident")
        nc.sync.dma_start(out=ident_t[:, :], in_=ident_d[:, :])
        cu_t = consts.tile([128, 32], U_DT, tag="cu", name="cu")           # uniform c=1/16
        nc.vector.memset(cu_t[:, :], 1.0 / K)
        gtm_t = consts.tile([128, 32], f32, tag="gtm", name="gtm")         # gt diag-block mask
        nc.vector.memset(gtm_t[0:64, 0:16], 1.0)
        nc.vector.memset(gtm_t[0:64, 16:32], 0.0)
        nc.vector.memset(gtm_t[64:128, 0:16], 0.0)
        nc.vector.memset(gtm_t[64:128, 16:32], 1.0)

        # ---- resident input copies ----
        nsubq = nch // 4  # un subtiles: 4 chunks x (4 pairs x 128) each
        un_t = [resident.tile([128, 2048], U_DT, tag=f"un{q}", name=f"un{q}")
                for q in range(nsubq)]
        ut8_t = [[resident.tile([128, subcols], UT_DT, tag=f"ut{p}_{q}",
                                name=f"ut{p}_{q}")
                  for q in range(nsub)] for p in range(NP)]
        cpc = subcols // CHUNK  # chunks per sub-tile

        def un_chunk(p, j):
            base = (j % 4) * 512 + p * CHUNK
            return un_t[j // 4][:, base:base + CHUNK]

        def ut8_chunk(p, j):
            return ut8_t[p][j // cpc][:, (j % cpc) * CHUNK:(j % cpc + 1) * CHUNK]

        # ---- persistent work tiles ----
        o_acc = work.tile([128, KD], f32, tag="oacc", name="oacc")      # masked output accum
        sm = work.tile([128, KD], f32, tag="sm", name="sm")
        sq = work.tile([128, KD], f32, tag="sq", name="sq")
        o_fin = work.tile([128, KD], f32, tag="ofin", name="ofin")
        t1_sb = work.tile([128, 128], U_DT, tag="t1", name="t1")      # Obd halves
        t2_sb = work.tile([128, 128], U_DT, tag="t2", name="t2")
        wop = [work.tile([128, 32], U_DT, tag=f"wop{p}", name=f"wop{p}")
               for p in range(NP)]
        gt_sb = [work.tile([128, 32], U_DT, tag=f"gts{p}", name=f"gts{p}")
                 for p in range(NP)]
        s2 = work.tile([128, 1], f32, tag="s2", name="s2")
        sc_a = work.tile([128, 1], f32, tag="sca", name="sca")
        sc_b = work.tile([128, 1], f32, tag="scb", name="scb")
        sc_e = work.tile([128, 1], f32, tag="sce", name="sce")
        xe = work.tile([128, 1], f32, tag="xe", name="xe")
        xh = work.tile([128, 1], f32, tag="xh", name="xh")
        yr = work.tile([128, 1], f32, tag="yr", name="yr")
        q1 = work.tile([128, 1], f32, tag="q1", name="q1")

        gt_tiles = [ps_gt.tile([128, 32], f32, tag=f"gt{p}", name=f"gt{p}",
                       padded_shape=[128, 512]) for p in range(NP)]

        # cross-batch blocks of gt_sb / wop stay zero for the whole kernel
        for p in range(NP):
            nc.vector.memset(gt_sb[p][0:64, 16:32], 0.0)
            nc.vector.memset(gt_sb[p][64:128, 0:16], 0.0)
            nc.vector.memset(wop[p][0:64, 16:32], 0.0)
            nc.vector.memset(wop[p][64:128, 0:16], 0.0)

        def routing_pass(it):
            """b-pass (if it>0) + softmax + G-pass, accumulating gt_tiles.

            Super-chunk-outer, pair-inner: consumes DMA tiles in order.
            The b-pass runs PIPE_AHEAD (s,p)-groups ahead of the G-pass so
            the PE never stalls on the ACT->DVE softmax latency of the
            current group (PE queues are strictly in-order)."""
            sp = ([(s, p) for s in range(nsup) for p in range(NP)] if sp_order
                  else [(s, p) for p in range(NP) for s in range(nsup)])
            uniform = it == 0 or "nobb" in ablate
            PIPE_AHEAD = 0 if uniform else pipe_ahead
            c_srcs = {}

            def emit_b(s, p):
                if uniform:
                    c_srcs[(s, p)] = lambda rel: cu_t[:, :]
                    return
                bb = ps_bb.tile([128, sup * 32], f32, tag="bb", name="bb",
                                padded_shape=[128, max(512, sup * 32)])
                for rel in range(sup):
                    j = s * sup + rel
                    nc.tensor.matmul(
                        bb[:, rel * 32:(rel + 1) * 32],
                        lhsT=ut8_chunk(p, j), rhs=wop[p][:, :],
                        start=(rel == 0), stop=(rel == sup - 1))
                if "nosm" in ablate:
                    c_srcs[(s, p)] = lambda rel: cu_t[:, :]
                    return
                e_t = e_pool.tile([128, sup * 32], U_DT, tag="e", name="e")
                nc.scalar.activation(e_t[:, :], bb[:, :], AFT.Exp)
                # bf16 z keeps every reduce operand 2-byte -> DVE 2x packed
                z_t = z_pool.tile([128, sup * 2], U_DT if z16 else f32,
                                  tag="z", name="z")
                with nc.allow_low_precision("z is a 16-term exp sum; c is "
                                            "bf16 downstream anyway"):
                    nc.vector.reduce_sum(
                        z_t[:, :].rearrange("p (a b) -> p a b", b=2),
                        e_t[:, :].rearrange("p (a b c) -> p a b c", b=2, c=K),
                        axis=AXT.X)
                zr_t = z_pool.tile([128, sup * 2], f32, tag="zr", name="zr")
                if z16:
                    # f32 staging for the approx reciprocal's bit-trick seed
                    # (cast on ACT -- it has slack, DVE is the busy engine)
                    zf_t = z_pool.tile([128, sup * 2], f32, tag="zf", name="zf")
                    nc.scalar.copy(zf_t[:, :], z_t[:, :])
                    nc.vector.reciprocal_approx_fast(zr_t[:, :], zf_t[:, :])
                else:
                    nc.vector.reciprocal(zr_t[:, :], z_t[:, :])
                c_t = c_pool.tile([128, sup * 32], U_DT, tag="c", name="c")
                nc.vector.tensor_mul(
                    c_t[:, :].rearrange("p (a b c) -> p a b c", b=2, c=K),
                    e_t[:, :].rearrange("p (a b c) -> p a b c", b=2, c=K),
                    zr_t[:, :].rearrange("p (a b) -> p a b", b=2)
                        .broadcast_to([128, sup, 2, K]))
                c_srcs[(s, p)] = lambda rel, c_t=c_t: \
                    c_t[:, rel * 32:(rel + 1) * 32]

            def emit_g(s, p):
                c_src = c_srcs.pop((s, p))
                for rel in range(sup):
                    j = s * sup + rel
                    if "nog" in ablate and j not in (0, nch - 1):
                        continue
                    nc.tensor.matmul(
                        gt_tiles[p][:, :],
                        lhsT=un_chunk(p, j), rhs=c_src(rel),
                        start=(j == 0), stop=(j == nch - 1))

            for i, (s, p) in enumerate(sp):
                emit_b(s, p)
                if i >= PIPE_AHEAD:
                    emit_g(*sp[i - PIPE_AHEAD])
            for i in range(len(sp) - PIPE_AHEAD, len(sp)):
                emit_g(*sp[i])

        def squash_scale():
            """sc_e = s2/(1+s2)/sqrt(s2+EPS), all on DVE (no ACT table)."""
            nc.vector.tensor_scalar_add(sc_a[:, :], s2[:, :], 1.0)
            nc.vector.reciprocal(sc_b[:, :], sc_a[:, :])
            nc.vector.tensor_scalar_add(xe[:, :], s2[:, :], EPS)
            nc.vector.tensor_scalar_mul(xh[:, :], xe[:, :], 0.5)
            # rsqrt(xe): bit-trick seed + 2 Newton steps
            yi = yr[:, :].bitcast(dt.int32)
            nc.vector.tensor_scalar(yi, xe[:, :].bitcast(dt.int32),
                                    1, -1, ALU.logical_shift_right,
                                    ALU.bitwise_xor)
            nc.vector.tensor_scalar_add(yi, yi, 0x5f3759e0)
            for _ in range(2):
                nc.vector.tensor_mul(q1[:, :], yr[:, :], yr[:, :])
                nc.vector.tensor_mul(q1[:, :], q1[:, :], xh[:, :])
                nc.vector.tensor_scalar(q1[:, :], q1[:, :], -1.0, 1.5,
                                        ALU.mult, ALU.add)
                nc.vector.tensor_mul(yr[:, :], yr[:, :], q1[:, :])
            nc.vector.tensor_mul(sc_e[:, :], sc_b[:, :], yr[:, :])
            nc.vector.tensor_mul(sc_e[:, :], sc_e[:, :], s2[:, :])

        def finalize(it):
            """gt -> sm -> mask -> squash -> (o_acc | o_fin); update Wo."""
            for p in range(NP):
                # keep only the in-batch diagonal blocks of GT-pair;
                # cross-batch blocks are garbage and must contract as zero
                nc.vector.tensor_mul(gt_sb[p][:, :], gt_tiles[p][:, :],
                                     gtm_t[:, :])
            for p in range(NP):
                sf = ps_bb.tile([32, KD], f32, tag="bb", name="sf",
                                padded_shape=[32, 512])
                nc.tensor.matmul(sf[:, :], lhsT=gt_sb[p][:, :],
                                 rhs=wsb_t[:, :], start=True, stop=True)
                # fused PSUM->SBUF copy + diagonal-block mask
                nc.vector.tensor_mul(sm[32 * p:32 * p + 32, :], sf[:, :],
                                     mask_t[32 * p:32 * p + 32, :])
            # s2 = rowsum(sm^2) on DVE (keeps ACT on the Exp table)
            nc.vector.tensor_mul(sq[:, :], sm[:, :], sm[:, :])
            nc.vector.reduce_sum(s2[:, :], sq[:, :], axis=AXT.X)
            squash_scale()
            tgt = o_fin if it == ROUTINGS - 1 else o_acc
            if it == 1:
                nc.vector.tensor_scalar_mul(o_fin[:, :], sm[:, :], sc_e[:, :])
                nc.vector.tensor_add(o_acc[:, :], o_acc[:, :], o_fin[:, :])
            else:
                nc.vector.tensor_scalar_mul(tgt[:, :], sm[:, :], sc_e[:, :])
            if it == ROUTINGS - 1:
                nc.sync.dma_start(out=out_d[:, :], in_=o_fin[:, :])
                return
            # Obd_b (256,16 block-diag of O_b) as columns of o_acc.T halves
            for h, t_sb in ((0, t1_sb), (1, t2_sb)):
                tp = ps_bb.tile([128, 128], f32, tag="bb", name="tp",
                                padded_shape=[128, 512])
                nc.tensor.transpose(tp[:, :], o_acc[:, h * 128:(h + 1) * 128],
                                    ident_t[:, :])
                nc.vector.tensor_copy(t_sb[:, :], tp[:, :])
            # Wo_b = W @ Obd_b, accumulated over the two 128-row halves of W.T
            wo = ps_bb.tile([64, NB * K], f32, tag="bb", name="wo",
                            padded_shape=[64, 512])
            for h2 in range(2):
                for b in range(NB):
                    nc.tensor.matmul(
                        wo[:, b * K:(b + 1) * K],
                        lhsT=wt_t[:, h2 * D:(h2 + 1) * D],
                        rhs=(t1_sb, t2_sb)[h2][:, b * K:(b + 1) * K],
                        start=(h2 == 0 and b == 0),
                        stop=(h2 == 1 and b == NB - 1))
            for b in range(NB):
                p, h = b // 2, b % 2
                nc.vector.tensor_copy(
                    wop[p][64 * h:64 * h + 64, 16 * h:16 * h + 16],
                    wo[:, b * K:(b + 1) * K])

        def rep_body():
            if "nodma" not in ablate:
                for q in range(nsubq):
                    nc.sync.dma_start(out=un_t[q][:, :], in_=un_d[q])
                for q in range(nsub):
                    for p in range(NP):
                        nc.sync.dma_start(
                            out=ut8_t[p][q][:, :],
                            in_=ut8_d[p, :, q * subcols:(q + 1) * subcols])
            if "dmawait" in ablate:
                # force completion: read one column of every DMA'd tile
                sink = work.tile([128, 64], f32, tag="sink", name="sink")
                for q in range(nsubq):
                    nc.vector.tensor_copy(sink[:, q:q + 1], un_t[q][:, 0:1])
                for p in range(NP):
                    for q in range(nsub):
                        nc.vector.tensor_copy(
                            sink[:, 32 + 4 * p + q:33 + 4 * p + q],
                            ut8_t[p][q][:, 0:1])
                nc.sync.dma_start(out=out_d[:, 0:64], in_=sink[:, :])
            elif "nocompute" not in ablate:
                for it in range(ROUTINGS):
                    routing_pass(it)
                    finalize(it)
            else:
                nc.vector.memset(o_fin[:, :], 0.0)
                nc.sync.dma_start(out=out_d[:, :], in_=o_fin[:, :])

        if "nodma" in ablate:
            for q in range(nsubq):
                nc.vector.memset(un_t[q][:, 0:2], 0.0)
            for p in range(NP):
                for q in range(nsub):
                    nc.vector.memset(ut8_t[p][q][:, 0:2], 0.0)
        if reps <= 33:
            for rep in range(reps):
                rep_body()
                if rep < reps - 1:
                    tc.strict_bb_all_engine_barrier()
        else:
            with tc.For_i(0, reps, 1):
                rep_body()

    nc.compile()
    return nc


def host_inputs(u_shard, W):
    """Per-core DRAM inputs from an (8, N, 64) f32 batch shard + W (64, 256)."""
    n = u_shard.shape[1]
    ut = np.ascontiguousarray(
        u_shard.reshape(NP, 2, n, D).transpose(0, 1, 3, 2).reshape(NP, 128, n))
    un = np.ascontiguousarray(
        u_shard.reshape(NP, 2, n // 512, 4, CHUNK, D)
        .transpose(2, 4, 3, 0, 1, 5).reshape(n // 512, 128, 2048)
    ).astype(U_NP)
    return {"ut8": ut.astype(U8_NP), "ut": ut.astype(U_NP), "un": un}


def host_consts(W):
    Wf = np.asarray(W, np.float32)
    wt = np.ascontiguousarray(Wf.T.reshape(2, 128, D)).astype(U_NP)
    wsb = np.ascontiguousarray(np.concatenate([Wf, Wf], 0)).astype(U_NP)
    base = np.kron(np.eye(K, dtype=np.float32), np.ones((1, DCAP), np.float32))
    mask = np.ascontiguousarray(np.tile(base, (NB, 1)))
    ident = np.eye(128, dtype=np.float32)
    return {"wt": wt, "wsb": wsb, "mask": mask, "ident": ident}


def extract_output(res_out):
    """(128, 256) masked f32 -> (8, 16, 16) squashed capsule outputs."""
    ar = np.arange(K)
    return res_out.reshape(NB, K, K, DCAP)[:, ar, ar, :]


_PROG_CACHE = {}


def _get_prog(n=N_FULL, reps=1):
    key = (n, reps)
    if key not in _PROG_CACHE:
        _PROG_CACHE[key] = build_program(n, reps)
    return _PROG_CACHE[key]


def kernel(u_vecs, W):
    u = np.ascontiguousarray(np.asarray(u_vecs, np.float32))
    assert u.shape == (B, N_FULL, D)
    nc = _get_prog()
    consts = host_consts(W)
    in_maps = [dict(consts, **host_inputs(u[c * NB:(c + 1) * NB], W))
               for c in range(NCORES)]
    res = run_bass_kernel_spmd(nc, in_maps, core_ids=list(range(NCORES)))
    return np.concatenate(
        [extract_output(res.results[c]["out"]) for c in range(NCORES)], axis=0
    ).astype(np.float32)
